# revision 1
# baseline (speedup 1.0000x reference)
"""Causal self-attention (B=4, L=2048, D=1024, H=16, HD=64) on 8 TRN2 cores.

Sharding: 8 shards = 4 batches x 2 head-groups (8 heads each). Each core:
  - QKV projection for its 8 heads (Q^T/K^T in [HD, L] layout, V in [L, HD])
  - causal attention per head, softmax without max-subtraction (logits are
    small by construction), row sums via a ones-column appended to V
  - partial output projection with its 512 rows of out_w
Host sums the two partials per batch and adds out_b.

All matmuls run in bf16 (fp32 PSUM accumulation); exp on ScalarE in fp32.
"""

import os

import numpy as np
import ml_dtypes

B, L, D, H, HD = 4, 2048, 1024, 16, 64
HPC = 8           # heads per core
NCORES = 8
QT_TILE = 512     # q columns per attention tile
NKB = L // 128    # key blocks of 128

_STATE = {}


def _build_nc(repeat=1):
    import concourse.bass as bass
    import concourse.mybir as mybir
    import concourse.tile as tile
    from concourse import bacc
    from concourse.masks import make_upper_triangular

    f32 = mybir.dt.float32
    bf16 = mybir.dt.bfloat16
    AF = mybir.ActivationFunctionType
    OP = mybir.AluOpType

    nc = bacc.Bacc(None, target_bir_lowering=False)

    xT = nc.dram_tensor("xT", [D, L], bf16, kind="ExternalInput")
    wqk = nc.dram_tensor("wqk", [D, 2 * HPC * HD], bf16, kind="ExternalInput")
    wv = nc.dram_tensor("wv", [D, HPC * HD], bf16, kind="ExternalInput")
    bqk = nc.dram_tensor("bqk", [128, 8], f32, kind="ExternalInput")
    bv = nc.dram_tensor("bv", [1, HPC * HD], bf16, kind="ExternalInput")
    w2 = nc.dram_tensor("w2", [HPC * HD, D], bf16, kind="ExternalInput")
    out = nc.dram_tensor("out", [L, D], f32, kind="ExternalOutput")

    KO = D // 128  # contraction blocks for the projections

    with tile.TileContext(nc) as tc:
        with (
            tc.tile_pool(name="const", bufs=1) as cpool,
            tc.tile_pool(name="weights", bufs=1) as wpool,
            tc.tile_pool(name="resident", bufs=1) as rpool,
            tc.tile_pool(name="xc", bufs=4) as xcpool,
            tc.tile_pool(name="ework", bufs=6) as epool,
            tc.tile_pool(name="ywork", bufs=3) as ypool,
            tc.tile_pool(name="rwork", bufs=4) as rwpool,
            tc.tile_pool(name="ps_mm", bufs=2, space="PSUM") as ps_mm,
            tc.tile_pool(name="ps_s", bufs=4, space="PSUM") as ps_s,
            tc.tile_pool(name="ps_u", bufs=1, space="PSUM") as ps_u,
        ):
            ones = cpool.tile([1, 128], bf16)
            nc.vector.memset(ones[:], 1.0)
            mask = cpool.tile([128, 128], bf16)
            make_upper_triangular(nc, mask[:], val=1.0, diag=True)

            wqk_sb = wpool.tile([128, KO, 2 * HPC * HD], bf16)
            nc.sync.dma_start(wqk_sb[:], wqk.rearrange("(ko p) m -> p ko m", p=128))
            wv_sb = wpool.tile([128, KO, HPC * HD], bf16)
            nc.sync.dma_start(wv_sb[:], wv.rearrange("(ko p) m -> p ko m", p=128))
            bqk_sb = wpool.tile([128, 8], f32)
            nc.sync.dma_start(bqk_sb[:], bqk[:])
            bv_sb = wpool.tile([1, HPC * HD], bf16)
            nc.sync.dma_start(bv_sb[:], bv[:])
            w2_sb = wpool.tile([128, 4, D], bf16)
            nc.sync.dma_start(w2_sb[:], w2.rearrange("(o p) n -> p o n", p=128))

            # Q^T / K^T packed as head pairs: head h lives at partitions
            # (h%2)*64..+64 of block h//2.
            QT = rpool.tile([128, 4, L], bf16)
            KT = rpool.tile([128, 4, L], bf16)
            # V with a ones column at index 64 (col 65 is alignment padding).
            V = rpool.tile([128, NKB, HPC, 66], bf16)
            nc.vector.memset(V[:, :, :, 64:66], 0.0)
            nc.vector.memset(V[:, :, :, 64:65], 1.0)
            OT = rpool.tile([128, 4, L], bf16)

            xTr = xT.rearrange("(ko p) n -> p ko n", p=128)

            def emit_pass():
              # ---- Phase A: projections ----
              for jt in range(L // 512):
                sl = slice(jt * 512, (jt + 1) * 512)
                xc = xcpool.tile([128, KO, 512], bf16)
                nc.sync.dma_start(xc[:], xTr[:, :, sl])
                # Q^T / K^T: out = wqk.T @ x^T
                for mb in range(8):
                    t = ps_mm.tile([128, 512], f32, tag="mm512")
                    for ko in range(KO):
                        nc.tensor.matmul(
                            t[:],
                            wqk_sb[:, ko, mb * 128:(mb + 1) * 128],
                            xc[:, ko, :],
                            start=(ko == 0),
                            stop=(ko == KO - 1),
                        )
                    dst = QT[:, mb, sl] if mb < 4 else KT[:, mb - 4, sl]
                    nc.vector.tensor_scalar_add(dst, t[:], bqk_sb[:, mb:mb + 1])
                # V: natural layout, bias folded in via a ones contraction row
                for qb in range(4):
                    g = jt * 4 + qb
                    tv = ps_mm.tile([128, 512], f32, tag="mm512")
                    for ko in range(KO):
                        nc.tensor.matmul(
                            tv[:],
                            xc[:, ko, qb * 128:(qb + 1) * 128],
                            wv_sb[:, ko, :],
                            start=(ko == 0),
                            stop=False,
                        )
                    nc.tensor.matmul(
                        tv[:], ones[0:1, :], bv_sb[0:1, :], start=False, stop=True
                    )
                    nc.vector.tensor_copy(
                        V[:, g, :, 0:64], tv.rearrange("p (h e) -> p h e", e=HD)
                    )

              # ---- Phase B: attention, one head pair at a time ----
              # Even head lives at partitions 0-63, odd head at 64-127: their
              # K=64 S^T matmuls target different PE row groups and overlap.
              for hp in range(4):
                  for jt in range(L // QT_TILE):
                      u_e = ps_u.tile([65, QT_TILE], f32, tag="u_e")
                      u_o = ps_u.tile([65, QT_TILE], f32, tag="u_o")
                      nkb = (jt + 1) * (QT_TILE // 128)
                      for kb in range(nkb):
                          q_off = max(0, kb * 128 - jt * QT_TILE)
                          qsl = slice(jt * QT_TILE + q_off, (jt + 1) * QT_TILE)
                          ksl = slice(kb * 128, (kb + 1) * 128)
                          s_e = ps_s.tile([128, QT_TILE], f32, tag="s_ps")
                          s_o = ps_s.tile([128, QT_TILE], f32, tag="s_ps")
                          nc.tensor.matmul(
                              s_e[:, q_off:], KT[0:64, hp, ksl], QT[0:64, hp, qsl],
                              start=True, stop=True,
                          )
                          nc.tensor.matmul(
                              s_o[:, q_off:], KT[64:128, hp, ksl], QT[64:128, hp, qsl],
                              start=True, stop=True,
                          )
                          et_e = epool.tile([128, QT_TILE], bf16, tag="et_e")
                          et_o = epool.tile([128, QT_TILE], bf16, tag="et_o")
                          nc.scalar.activation(
                              et_e[:, q_off:], s_e[:, q_off:], AF.Exp, scale=0.125)
                          nc.scalar.activation(
                              et_o[:, q_off:], s_o[:, q_off:], AF.Exp, scale=0.125)
                          if kb * 128 >= jt * QT_TILE:  # diagonal block
                              for et in (et_e, et_o):
                                  nc.vector.tensor_tensor(
                                      out=et[:, q_off:q_off + 128],
                                      in0=et[:, q_off:q_off + 128],
                                      in1=mask[:],
                                      op=OP.mult,
                                  )
                          nc.tensor.matmul(
                              u_e[:, q_off:], V[:, kb, 2 * hp, 0:65], et_e[:, q_off:],
                              start=(kb == 0), stop=(kb == nkb - 1),
                          )
                          nc.tensor.matmul(
                              u_o[:, q_off:], V[:, kb, 2 * hp + 1, 0:65], et_o[:, q_off:],
                              start=(kb == 0), stop=(kb == nkb - 1),
                          )
                      # normalize: O^T = U^T * (1/rowsum) broadcast via PE
                      sl = slice(jt * QT_TILE, (jt + 1) * QT_TILE)
                      for side, u_ps in ((0, u_e), (1, u_o)):
                          u_sb = rwpool.tile([65, QT_TILE], f32, tag="u_sb")
                          nc.scalar.copy(u_sb[:], u_ps[:])
                          rcp = rwpool.tile([1, QT_TILE], f32)
                          nc.vector.reciprocal(rcp[:], u_sb[64:65, :])
                          rcpb = rwpool.tile([1, QT_TILE], bf16)
                          nc.vector.tensor_copy(rcpb[:], rcp[:])
                          b_ps = ps_mm.tile([128, QT_TILE], f32, tag="mm512")
                          nc.tensor.matmul(
                              b_ps[0:64, :], ones[0:1, 0:64], rcpb[0:1, :],
                              start=True, stop=True,
                          )
                          b_sb = rwpool.tile([64, QT_TILE], f32, tag="bcast_sb")
                          nc.scalar.copy(b_sb[:], b_ps[0:64, :])
                          nc.vector.tensor_tensor(
                              out=OT[side * 64:side * 64 + 64, hp, sl],
                              in0=u_sb[0:64, :],
                              in1=b_sb[:],
                              op=OP.mult,
                          )

              # ---- Phase C: output projection (partial: this core's 512 rows) ----
              for qb in range(L // 128):
                  for nb in range(D // 512):
                      y_ps = ps_mm.tile([128, 512], f32, tag="mm512")
                      for hp in range(4):
                          nc.tensor.matmul(
                              y_ps[:],
                              OT[:, hp, qb * 128:(qb + 1) * 128],
                              w2_sb[:, hp, nb * 512:(nb + 1) * 512],
                              start=(hp == 0),
                              stop=(hp == 3),
                          )
                      y_sb = ypool.tile([128, 512], f32)
                      nc.vector.tensor_copy(y_sb[:], y_ps[:])
                      nc.sync.dma_start(
                          out[qb * 128:(qb + 1) * 128, nb * 512:(nb + 1) * 512], y_sb[:]
                      )


            for _rep in range(repeat):
                emit_pass()
    nc.compile()
    return nc


def _get_nc():
    if "nc" not in _STATE:
        _STATE["nc"] = _build_nc()
    return _STATE["nc"]


def kernel(x, in_w, in_b, out_w, out_b):
    from concourse.bass_utils import run_bass_kernel_spmd

    bf = ml_dtypes.bfloat16
    x = np.asarray(x, dtype=np.float32)
    in_w = np.asarray(in_w, dtype=np.float32)
    in_b = np.asarray(in_b, dtype=np.float32)
    out_w = np.asarray(out_w, dtype=np.float32)
    out_b = np.asarray(out_b, dtype=np.float32)

    nc = _get_nc()

    in_maps = []
    for c in range(NCORES):
        b, hg = c // 2, c % 2
        hsl = slice(hg * HPC * HD, (hg + 1) * HPC * HD)  # 512 cols of each section
        wq = in_w[:, 0:D][:, hsl]
        wk = in_w[:, D:2 * D][:, hsl]
        wv_ = in_w[:, 2 * D:3 * D][:, hsl]
        bq = in_b[0:D][hsl]
        bk = in_b[D:2 * D][hsl]
        bv_ = in_b[2 * D:3 * D][hsl]
        in_maps.append({
            "xT": np.ascontiguousarray(x[b].T).astype(bf),
            "wqk": np.ascontiguousarray(
                np.concatenate([wq, wk], axis=1)).astype(bf),
            "wv": np.ascontiguousarray(wv_).astype(bf),
            "bqk": np.ascontiguousarray(
                np.concatenate([bq, bk]).reshape(8, 128).T).astype(np.float32),
            "bv": np.ascontiguousarray(bv_.reshape(1, -1)).astype(bf),
            "w2": np.ascontiguousarray(out_w[hsl, :]).astype(bf),
        })

    trace = bool(int(os.environ.get("KERNEL_TRACE", "0")))
    if not trace:
        # the axon NTFF profile hook is absent in this container; make sure a
        # stray BASS_TRACE=1 in the environment can't route us into it
        os.environ["BASS_NEVER_TRACE"] = "1"
    res = run_bass_kernel_spmd(
        nc, in_maps, core_ids=list(range(NCORES)), trace=trace,
    )
    _STATE["last_result"] = res
    _STATE["last_in_maps"] = in_maps

    y = np.zeros((B, L, D), dtype=np.float32)
    for c in range(NCORES):
        y[c // 2] += res.results[c]["out"]
    y += out_b[None, None, :]
    return y



# revision 5
# speedup vs baseline: 1.5888x; 1.5888x over previous
"""Causal self-attention (B=4, L=2048, D=1024, H=16, HD=64) on 8 TRN2 cores.

Sharding: 8 shards = 4 batches x 2 head-groups (8 heads each). Each core:
  - QKV projection for its 8 heads (Q^T/K^T in [HD, L] layout, V in [L, HD])
  - causal attention per head pair (even head on partitions 0-63, odd on
    64-127), softmax without max-subtraction (logits are small by
    construction), row sums via a ones-column appended to V
  - partial output projection with its 512 rows of out_w
Host sums the two partials per batch and adds out_b.

Single fused pipeline per 512-query tile jt:
  S^T = K^T q  (PSUM, causal mask folded in as an identity x (-3000 *
  strict-lower) matmul joining the accumulation), one exp per key block
  covering both heads of the pair, then U = E^T V_aug in [q, d] layout
  (N=65 with the ones column), per-partition normalize on DVE, PE
  transpose back to [d, q] for the output projection.

Projection / output-projection work is emitted as filler thunks between
attention key blocks so the PE never idles (the PE clock drops ~2x after
any idle gap). K^T and V are double-buffered across repeat passes so
consecutive passes in the timing NEFF overlap.
"""

import os
from collections import deque

import numpy as np
import ml_dtypes

B, L, D, H, HD = 4, 2048, 1024, 16, 64
HPC = 8           # heads per core
NCORES = 8
NKB = L // 128    # key blocks of 128
NJT = L // 512    # query tiles of 512

_STATE = {}


def _build_nc(repeat=1):
    import concourse.bass as bass
    import concourse.mybir as mybir
    import concourse.tile as tile
    from concourse import bacc
    from concourse.masks import make_lower_triangular, make_identity

    f32 = mybir.dt.float32
    bf16 = mybir.dt.bfloat16
    AF = mybir.ActivationFunctionType

    nc = bacc.Bacc(None, target_bir_lowering=False)

    xT = nc.dram_tensor("xT", [D, L], bf16, kind="ExternalInput")
    wqk = nc.dram_tensor("wqk", [D, 2 * HPC * HD], bf16, kind="ExternalInput")
    wv = nc.dram_tensor("wv", [D, HPC * HD], bf16, kind="ExternalInput")
    bqk = nc.dram_tensor("bqk", [128, 8], f32, kind="ExternalInput")
    bv = nc.dram_tensor("bv", [1, HPC * HD], bf16, kind="ExternalInput")
    w2 = nc.dram_tensor("w2", [HPC * HD, D], bf16, kind="ExternalInput")
    out = nc.dram_tensor("out", [L, D], f32, kind="ExternalOutput")
    debug = bool(int(os.environ.get("KDEBUG", "0")))
    if debug:
        qt_d = nc.dram_tensor("qt_d", [128, 4 * L], bf16, kind="ExternalOutput")
        kt_d = nc.dram_tensor("kt_d", [128, 4 * L], bf16, kind="ExternalOutput")
        v_d = nc.dram_tensor("v_d", [128, NKB * HPC * 66], bf16,
                             kind="ExternalOutput")
        ot_d = nc.dram_tensor("ot_d", [128, 4 * L], bf16, kind="ExternalOutput")
        et_d = nc.dram_tensor("et_d", [128, 2 * 512], bf16, kind="ExternalOutput")

    KO = D // 128  # contraction blocks for the projections
    xTr = xT.rearrange("(ko p) n -> p ko n", p=128)

    with tile.TileContext(nc) as tc:
        with (
            tc.tile_pool(name="const", bufs=1) as cpool,
            tc.tile_pool(name="weights", bufs=1) as wpool,
            tc.tile_pool(name="resident", bufs=1) as rpool,
            tc.tile_pool(name="xc", bufs=2) as xcpool,
            tc.tile_pool(name="ework", bufs=1) as epool,
            tc.tile_pool(name="nwork", bufs=3) as npool,
            tc.tile_pool(name="ywork", bufs=3) as ypool,
            tc.tile_pool(name="ps_s", bufs=2, space="PSUM") as ps_s,
            tc.tile_pool(name="ps_u", bufs=2, space="PSUM") as ps_u,
            tc.tile_pool(name="ps_mm", bufs=2, space="PSUM") as ps_mm,
        ):
            ones = cpool.tile([1, 128], bf16)
            nc.vector.memset(ones[:], 1.0)
            ident = cpool.tile([128, 128], bf16)
            make_identity(nc, ident[:])
            maskneg = cpool.tile([128, 128], bf16)
            make_lower_triangular(nc, maskneg[:], val=-3000.0, diag=False)

            wqk_sb = wpool.tile([128, KO, 2 * HPC * HD], bf16)
            nc.sync.dma_start(wqk_sb[:], wqk.rearrange("(ko p) m -> p ko m", p=128))
            wv_sb = wpool.tile([128, KO, HPC * HD], bf16)
            nc.sync.dma_start(wv_sb[:], wv.rearrange("(ko p) m -> p ko m", p=128))
            bqk_sb = wpool.tile([128, 8], f32)
            nc.sync.dma_start(bqk_sb[:], bqk[:])
            bv_sb = wpool.tile([1, HPC * HD], bf16)
            nc.sync.dma_start(bv_sb[:], bv[:])
            w2_sb = wpool.tile([128, 4, D], bf16)
            nc.sync.dma_start(w2_sb[:], w2.rearrange("(o p) n -> p o n", p=128))

            # Q^T packed as head pairs: head h at partitions (h%2)*64..+64 of
            # block h//2.  K^T and V double-buffered by pass parity so the
            # next pass's projections can overlap this pass's last strips.
            QT = rpool.tile([128, 4, L], bf16)
            OT = rpool.tile([128, 4, L], bf16)
            KTbuf = [rpool.tile([128, 4, L], bf16, name=f"KT{i}") for i in (0, 1)]
            # V with a ones column at index 64 (col 65 is alignment padding).
            Vbuf = [rpool.tile([128, NKB, HPC, 66], bf16, name=f"V{i}")
                    for i in (0, 1)]
            for Vb in Vbuf:
                nc.vector.memset(Vb[:, :, :, 64:66], 0.0)
                nc.vector.memset(Vb[:, :, :, 64:65], 1.0)

            et_tiles = [
                epool.tile([128, 2, 512], bf16, tag=f"et{kb}", name=f"et{kb}")
                for kb in range(NKB)
            ]

            fillers = deque()

            def pop_filler(n=1):
                for _ in range(n):
                    if fillers:
                        fillers.popleft()()

            def issue_xc_dma(jt):
                xc = xcpool.tile([128, KO, 512], bf16, tag="xc", name="xc")
                nc.sync.dma_start(xc[:], xTr[:, :, jt * 512:(jt + 1) * 512])
                return xc

            def proj_thunks(jt, par, xc):
                """QKV projection for query/key tile jt into parity-par bufs."""
                KT, V = KTbuf[par], Vbuf[par]
                sl = slice(jt * 512, (jt + 1) * 512)
                thunks = []

                def qk(mb):
                    def t():
                        tq = ps_mm.tile([128, 512], f32, tag="mm512", name="tq")
                        for ko in range(KO):
                            nc.tensor.matmul(
                                tq[:],
                                wqk_sb[:, ko, mb * 128:(mb + 1) * 128],
                                xc[:, ko, :],
                                start=(ko == 0),
                                stop=(ko == KO - 1),
                            )
                        dst = QT[:, mb, sl] if mb < 4 else KT[:, mb - 4, sl]
                        nc.vector.tensor_scalar_add(dst, tq[:], bqk_sb[:, mb:mb + 1])
                    return t

                def vproj(qb):
                    def t():
                        tv = ps_mm.tile([128, 512], f32, tag="mm512", name="tv")
                        for ko in range(KO):
                            nc.tensor.matmul(
                                tv[:],
                                xc[:, ko, qb * 128:(qb + 1) * 128],
                                wv_sb[:, ko, :],
                                start=(ko == 0),
                                stop=False,
                            )
                        nc.tensor.matmul(
                            tv[:], ones[0:1, :], bv_sb[0:1, :],
                            start=False, stop=True,
                        )
                        nc.vector.tensor_copy(
                            V[:, jt * 4 + qb, :, 0:64],
                            tv.rearrange("p (h e) -> p h e", e=HD),
                        )
                    return t

                for mb in range(8):
                    thunks.append(qk(mb))
                for qb in range(4):
                    thunks.append(vproj(qb))
                return thunks

            def outproj_thunks(jt):
                """Output projection for the 512 queries of tile jt."""
                thunks = []

                def yproj(qb, nb):
                    def t():
                        y = ps_mm.tile([128, 512], f32, tag="mm512", name="y")
                        for hp in range(4):
                            nc.tensor.matmul(
                                y[:],
                                OT[:, hp, qb * 128:(qb + 1) * 128],
                                w2_sb[:, hp, nb * 512:(nb + 1) * 512],
                                start=(hp == 0),
                                stop=(hp == 3),
                            )
                        ysb = ypool.tile([128, 512], f32, tag="ysb", name="ysb")
                        nc.vector.tensor_copy(ysb[:], y[:])
                        nc.sync.dma_start(
                            out[qb * 128:(qb + 1) * 128, nb * 512:(nb + 1) * 512],
                            ysb[:],
                        )
                    return t

                for qb in range(jt * 4, (jt + 1) * 4):
                    for nb in range(2):
                        thunks.append(yproj(qb, nb))
                return thunks

            def strip(hp, jt, par):
                """Attention for head pair hp, query tile jt."""
                KT, V = KTbuf[par], Vbuf[par]
                nkb = 4 * (jt + 1)
                # S + exp phase
                for kb in range(nkb):
                    qoff = max(0, (kb - 4 * jt) * 128)
                    qsl = slice(jt * 512 + qoff, (jt + 1) * 512)
                    ksl = slice(kb * 128, (kb + 1) * 128)
                    diag = kb >= 4 * jt
                    sp = ps_s.tile([128, 2, 512], f32, tag="sp", name="sp")
                    for side in (0, 1):
                        p0, p1 = side * 64, side * 64 + 64
                        nc.tensor.matmul(
                            sp[:, side, qoff:],
                            KT[p0:p1, hp, ksl],
                            QT[p0:p1, hp, qsl],
                            start=True,
                            stop=not diag,
                        )
                        if diag:
                            nc.tensor.matmul(
                                sp[:, side, qoff:qoff + 128],
                                ident[:],
                                maskneg[:],
                                start=False,
                                stop=True,
                            )
                    nc.scalar.activation(
                        et_tiles[kb][:, :, qoff:], sp[:, :, qoff:],
                        AF.Exp, scale=0.125,
                    )
                    pop_filler()
                # U phase: [q, d] layout, ones column gives row sums at col 64
                for qc in range(4):
                    last = 4 * jt + qc
                    tp = ps_mm.tile([128, 128], bf16, tag="mm512", name="tp")
                    for side in (0, 1):
                        u = ps_u.tile([128, 66], f32, tag="u", name="u")
                        for kb in range(last + 1):
                            nc.tensor.matmul(
                                u[:, 0:65],
                                et_tiles[kb][:, side, qc * 128:(qc + 1) * 128],
                                V[:, kb, 2 * hp + side, 0:65],
                                start=(kb == 0),
                                stop=(kb == last),
                            )
                        rcp = npool.tile([128, 1], f32, tag="rcp", name="rcp")
                        nc.vector.reciprocal(rcp[:], u[:, 64:65])
                        oqd = npool.tile([128, 64], bf16, tag="oqd", name="oqd")
                        nc.vector.tensor_scalar_mul(oqd[:], u[:, 0:64], rcp[:])
                        nc.tensor.transpose(
                            tp[side * 64:side * 64 + 64, :], oqd[:], ident[:],
                            tile_position=(0, side * 64),
                        )
                    csl = slice(jt * 512 + qc * 128, jt * 512 + (qc + 1) * 128)
                    nc.vector.tensor_copy(OT[:, hp, csl], tp[:])
                    pop_filler()

            # ---- emit passes ----
            for p in range(repeat):
                par = p % 2
                if p == 0:
                    xc0 = issue_xc_dma(0)
                    for t in proj_thunks(0, 0, xc0):
                        t()
                for jt in range(NJT):
                    if jt + 1 < NJT:
                        xcn = issue_xc_dma(jt + 1)
                        fillers.extend(proj_thunks(jt + 1, par, xcn))
                    elif p + 1 < repeat:
                        xcn = issue_xc_dma(0)
                        fillers.extend(proj_thunks(0, (p + 1) % 2, xcn))
                    if jt >= 1:
                        fillers.extend(outproj_thunks(jt - 1))
                    elif p >= 1:
                        fillers.extend(outproj_thunks(3))
                    for hp in range(4):
                        strip(hp, jt, par)
            # tail: remaining projections of the last pass
            while fillers:
                fillers.popleft()()
            for t in outproj_thunks(2):
                t()
            for t in outproj_thunks(3):
                t()
            if debug:
                nc.sync.dma_start(qt_d[:].rearrange("p (a b) -> p a b", a=4),
                                  QT[:])
                nc.sync.dma_start(kt_d[:].rearrange("p (a b) -> p a b", a=4),
                                  KTbuf[(repeat - 1) % 2][:])
                nc.sync.dma_start(
                    v_d[:].rearrange("p (a b c) -> p a b c", a=NKB, b=HPC),
                    Vbuf[(repeat - 1) % 2][:])
                nc.sync.dma_start(ot_d[:].rearrange("p (a b) -> p a b", a=4),
                                  OT[:])
                nc.sync.dma_start(et_d[:].rearrange("p (a b) -> p a b", a=2),
                                  et_tiles[0][:])

    nc.compile()
    return nc


def _get_nc():
    if "nc" not in _STATE:
        _STATE["nc"] = _build_nc()
    return _STATE["nc"]


def kernel(x, in_w, in_b, out_w, out_b):
    from concourse.bass_utils import run_bass_kernel_spmd

    bf = ml_dtypes.bfloat16
    x = np.asarray(x, dtype=np.float32)
    in_w = np.asarray(in_w, dtype=np.float32)
    in_b = np.asarray(in_b, dtype=np.float32)
    out_w = np.asarray(out_w, dtype=np.float32)
    out_b = np.asarray(out_b, dtype=np.float32)

    nc = _get_nc()

    in_maps = []
    for c in range(NCORES):
        b, hg = c // 2, c % 2
        hsl = slice(hg * HPC * HD, (hg + 1) * HPC * HD)  # 512 cols of each section
        wq = in_w[:, 0:D][:, hsl]
        wk = in_w[:, D:2 * D][:, hsl]
        wv_ = in_w[:, 2 * D:3 * D][:, hsl]
        bq = in_b[0:D][hsl]
        bk = in_b[D:2 * D][hsl]
        bv_ = in_b[2 * D:3 * D][hsl]
        in_maps.append({
            "xT": np.ascontiguousarray(x[b].T).astype(bf),
            "wqk": np.ascontiguousarray(
                np.concatenate([wq, wk], axis=1)).astype(bf),
            "wv": np.ascontiguousarray(wv_).astype(bf),
            "bqk": np.ascontiguousarray(
                np.concatenate([bq, bk]).reshape(8, 128).T).astype(np.float32),
            "bv": np.ascontiguousarray(bv_.reshape(1, -1)).astype(bf),
            "w2": np.ascontiguousarray(out_w[hsl, :]).astype(bf),
        })

    trace = bool(int(os.environ.get("KERNEL_TRACE", "0")))
    if not trace:
        # the axon NTFF profile hook is absent in this container; make sure a
        # stray BASS_TRACE=1 in the environment can't route us into it
        os.environ["BASS_NEVER_TRACE"] = "1"
    res = run_bass_kernel_spmd(
        nc, in_maps, core_ids=list(range(NCORES)), trace=trace,
    )
    _STATE["last_result"] = res
    _STATE["last_in_maps"] = in_maps

    y = np.zeros((B, L, D), dtype=np.float32)
    for c in range(NCORES):
        y[c // 2] += res.results[c]["out"]
    y += out_b[None, None, :]
    return y


# revision 8
# speedup vs baseline: 1.9621x; 1.2350x over previous
"""Causal self-attention (B=4, L=2048, D=1024, H=16, HD=64) on 8 TRN2 cores.

Sharding: 8 shards = 4 batches x 2 head-groups (8 heads each). Each core:
  - QKV projection for its 8 heads (Q^T/K^T in [HD, L] layout, V in [L, HD])
  - causal attention per head pair (even head on partitions 0-63, odd on
    64-127), softmax without max-subtraction (logits are small by
    construction), row sums via a ones-column appended to V
  - partial output projection with its 512 rows of out_w
Host sums the two partials per batch and adds out_b.

Single fused pipeline per 512-query tile jt:
  S^T = K^T q  (PSUM, causal mask folded in as an identity x (-3000 *
  strict-lower) matmul joining the accumulation), one exp per key block
  covering both heads of the pair, then U = E^T V_aug in [q, d] layout
  (N=65 with the ones column), per-partition normalize on DVE, PE
  transpose back to [d, q] for the output projection.

Projection / output-projection work is emitted as filler thunks between
attention key blocks so the PE never idles (the PE clock drops ~2x after
any idle gap). K^T and V are double-buffered across repeat passes so
consecutive passes in the timing NEFF overlap.
"""

import os
from collections import deque

import numpy as np
import ml_dtypes

B, L, D, H, HD = 4, 2048, 1024, 16, 64
HPC = 8           # heads per core
NCORES = 8
NKB = L // 128    # key blocks of 128
NJT = L // 512    # query tiles of 512

_STATE = {}


def _build_nc(repeat=1):
    import concourse.bass as bass
    import concourse.mybir as mybir
    import concourse.tile as tile
    from concourse import bacc
    from concourse.masks import make_lower_triangular, make_identity

    f32 = mybir.dt.float32
    bf16 = mybir.dt.bfloat16
    AF = mybir.ActivationFunctionType

    nc = bacc.Bacc(None, target_bir_lowering=False)

    xT = nc.dram_tensor("xT", [D, L], bf16, kind="ExternalInput")
    wqk = nc.dram_tensor("wqk", [D, 2 * HPC * HD], bf16, kind="ExternalInput")
    wv = nc.dram_tensor("wv", [D, HPC * HD], bf16, kind="ExternalInput")
    bqk = nc.dram_tensor("bqk", [128, 8], f32, kind="ExternalInput")
    bv = nc.dram_tensor("bv", [1, HPC * HD], bf16, kind="ExternalInput")
    w2 = nc.dram_tensor("w2", [HPC * HD, D], bf16, kind="ExternalInput")
    out = nc.dram_tensor("out", [L, D], f32, kind="ExternalOutput")
    debug = bool(int(os.environ.get("KDEBUG", "0")))
    if debug:
        qt_d = nc.dram_tensor("qt_d", [128, 4 * L], bf16, kind="ExternalOutput")
        kt_d = nc.dram_tensor("kt_d", [128, 4 * L], bf16, kind="ExternalOutput")
        v_d = nc.dram_tensor("v_d", [128, NKB * HPC * 66], bf16,
                             kind="ExternalOutput")
        ot_d = nc.dram_tensor("ot_d", [128, 4 * L], bf16, kind="ExternalOutput")
        et_d = nc.dram_tensor("et_d", [128, 2 * 512], bf16, kind="ExternalOutput")

    KO = D // 128  # contraction blocks for the projections
    xTr = xT.rearrange("(ko p) n -> p ko n", p=128)

    with tile.TileContext(nc) as tc:
        with (
            tc.tile_pool(name="const", bufs=1) as cpool,
            tc.tile_pool(name="weights", bufs=1) as wpool,
            tc.tile_pool(name="resident", bufs=1) as rpool,
            tc.tile_pool(name="xc", bufs=2) as xcpool,
            tc.tile_pool(name="ework", bufs=1) as epool,
            tc.tile_pool(name="nwork", bufs=3) as npool,
            tc.tile_pool(name="ywork", bufs=3) as ypool,
            tc.tile_pool(name="ps_s", bufs=2, space="PSUM") as ps_s,
            tc.tile_pool(name="ps_u", bufs=2, space="PSUM") as ps_u,
            tc.tile_pool(name="ps_mm", bufs=2, space="PSUM") as ps_mm,
        ):
            ones = cpool.tile([1, 128], bf16)
            nc.vector.memset(ones[:], 1.0)
            ident = cpool.tile([128, 128], bf16)
            make_identity(nc, ident[:])
            maskneg = cpool.tile([128, 128], bf16)
            make_lower_triangular(nc, maskneg[:], val=-3000.0, diag=False)

            wqk_sb = wpool.tile([128, KO, 2 * HPC * HD], bf16)
            nc.sync.dma_start(wqk_sb[:], wqk.rearrange("(ko p) m -> p ko m", p=128))
            wv_sb = wpool.tile([128, KO, HPC * HD], bf16)
            nc.sync.dma_start(wv_sb[:], wv.rearrange("(ko p) m -> p ko m", p=128))
            bqk_sb = wpool.tile([128, 8], f32)
            nc.sync.dma_start(bqk_sb[:], bqk[:])
            bv_sb = wpool.tile([1, HPC * HD], bf16)
            nc.sync.dma_start(bv_sb[:], bv[:])
            w2_sb = wpool.tile([128, 4, D], bf16)
            nc.sync.dma_start(w2_sb[:], w2.rearrange("(o p) n -> p o n", p=128))

            # Q^T packed as head pairs: head h at partitions (h%2)*64..+64 of
            # block h//2.  K^T and V double-buffered by pass parity so the
            # next pass's projections can overlap this pass's last strips.
            QT = rpool.tile([128, 4, L], bf16)
            OT = rpool.tile([128, 4, L], bf16)
            KTbuf = [rpool.tile([128, 4, L], bf16, name=f"KT{i}") for i in (0, 1)]
            # V with a ones column at index 64 (col 65 is alignment padding).
            Vbuf = [rpool.tile([128, NKB, HPC, 66], bf16, name=f"V{i}")
                    for i in (0, 1)]
            for Vb in Vbuf:
                nc.vector.memset(Vb[:, :, :, 64:66], 0.0)
                nc.vector.memset(Vb[:, :, :, 64:65], 1.0)

            et_tiles = [
                epool.tile([128, 2, 512], bf16, tag=f"et{kb}", name=f"et{kb}")
                for kb in range(NKB)
            ]

            fillers = deque()
            pace = {"credit": 0.0, "rate": 0.0}

            def set_pace(slots):
                pace["rate"] = len(fillers) / max(1, slots)
                pace["credit"] = 0.0

            def pop_filler():
                """Paced pop: spread the queued thunks over the set's slots."""
                pace["credit"] += pace["rate"]
                while pace["credit"] >= 1.0 and fillers:
                    pace["credit"] -= 1.0
                    fillers.popleft()()

            def issue_xc_dma(jt):
                xc = xcpool.tile([128, KO, 512], bf16, tag="xc", name="xc")
                nc.sync.dma_start(xc[:], xTr[:, :, jt * 512:(jt + 1) * 512])
                return xc

            def proj_thunks(jt, par, xc):
                """QKV projection for query/key tile jt into parity-par bufs."""
                KT, V = KTbuf[par], Vbuf[par]
                sl = slice(jt * 512, (jt + 1) * 512)
                thunks = []

                def qk(mb):
                    def t():
                        tq = ps_mm.tile([128, 512], f32, tag="mm512", name="tq")
                        for ko in range(KO):
                            nc.tensor.matmul(
                                tq[:],
                                wqk_sb[:, ko, mb * 128:(mb + 1) * 128],
                                xc[:, ko, :],
                                start=(ko == 0),
                                stop=(ko == KO - 1),
                            )
                        dst = QT[:, mb, sl] if mb < 4 else KT[:, mb - 4, sl]
                        nc.vector.tensor_scalar_add(dst, tq[:], bqk_sb[:, mb:mb + 1])
                    return t

                def vproj(qb):
                    def t():
                        tv = ps_mm.tile([128, 512], f32, tag="mm512", name="tv")
                        for ko in range(KO):
                            nc.tensor.matmul(
                                tv[:],
                                xc[:, ko, qb * 128:(qb + 1) * 128],
                                wv_sb[:, ko, :],
                                start=(ko == 0),
                                stop=False,
                            )
                        nc.tensor.matmul(
                            tv[:], ones[0:1, :], bv_sb[0:1, :],
                            start=False, stop=True,
                        )
                        nc.vector.tensor_copy(
                            V[:, jt * 4 + qb, :, 0:64],
                            tv.rearrange("p (h e) -> p h e", e=HD),
                        )
                    return t

                for mb in range(8):
                    thunks.append(qk(mb))
                for qb in range(4):
                    thunks.append(vproj(qb))
                return thunks

            def outproj_thunks(jt):
                """Output projection for the 512 queries of tile jt."""
                thunks = []

                def yproj(qb, nb):
                    def t():
                        y = ps_mm.tile([128, 512], f32, tag="mm512", name="y")
                        for hp in range(4):
                            nc.tensor.matmul(
                                y[:],
                                OT[:, hp, qb * 128:(qb + 1) * 128],
                                w2_sb[:, hp, nb * 512:(nb + 1) * 512],
                                start=(hp == 0),
                                stop=(hp == 3),
                            )
                        ysb = ypool.tile([128, 512], f32, tag="ysb", name="ysb")
                        nc.vector.tensor_copy(ysb[:], y[:])
                        nc.sync.dma_start(
                            out[qb * 128:(qb + 1) * 128, nb * 512:(nb + 1) * 512],
                            ysb[:],
                        )
                    return t

                for qb in range(jt * 4, (jt + 1) * 4):
                    for nb in range(2):
                        thunks.append(yproj(qb, nb))
                return thunks

            def strip(hp, jt, par):
                """Attention for head pair hp, query tile jt."""
                KT, V = KTbuf[par], Vbuf[par]
                nkb = 4 * (jt + 1)
                # S + exp phase
                for kb in range(nkb):
                    qoff = max(0, (kb - 4 * jt) * 128)
                    qsl = slice(jt * 512 + qoff, (jt + 1) * 512)
                    ksl = slice(kb * 128, (kb + 1) * 128)
                    diag = kb >= 4 * jt
                    sp = ps_s.tile([128, 2, 512], f32, tag="sp", name="sp")
                    for side in (0, 1):
                        p0, p1 = side * 64, side * 64 + 64
                        nc.tensor.matmul(
                            sp[:, side, qoff:],
                            KT[p0:p1, hp, ksl],
                            QT[p0:p1, hp, qsl],
                            start=True,
                            stop=not diag,
                        )
                        if diag:
                            nc.tensor.matmul(
                                sp[:, side, qoff:qoff + 128],
                                ident[:],
                                maskneg[:],
                                start=False,
                                stop=True,
                            )
                    nc.scalar.activation(
                        et_tiles[kb][:, :, qoff:], sp[:, :, qoff:],
                        AF.Exp, scale=0.125,
                    )
                    pop_filler()
                # U phase: [q, d] layout, ones column gives row sums at col 64.
                # The DVE normalize runs while the PE continues with the next
                # chunk's U matmuls; transposes are deferred to the end so the
                # PE never waits on the DVE chain.
                oqds = []
                for qc in range(4):
                    last = 4 * jt + qc
                    for side in (0, 1):
                        u = ps_u.tile([128, 66], f32, tag="u", name="u")
                        for kb in range(last + 1):
                            nc.tensor.matmul(
                                u[:, 0:65],
                                et_tiles[kb][:, side, qc * 128:(qc + 1) * 128],
                                V[:, kb, 2 * hp + side, 0:65],
                                start=(kb == 0),
                                stop=(kb == last),
                            )
                        rcp = npool.tile([128, 1], f32, tag="rcp", name="rcp")
                        nc.vector.reciprocal(rcp[:], u[:, 64:65])
                        oqd = npool.tile([128, 64], bf16, tag="oqd",
                                         bufs=8, name="oqd")
                        nc.vector.tensor_scalar_mul(oqd[:], u[:, 0:64], rcp[:])
                        oqds.append(oqd)
                    pop_filler()
                for qc in range(4):
                    tp = ps_mm.tile([128, 128], bf16, tag="mm512", name="tp")
                    for side in (0, 1):
                        nc.tensor.transpose(
                            tp[side * 64:side * 64 + 64, :],
                            oqds[2 * qc + side][:], ident[:],
                            tile_position=(0, side * 64),
                        )
                    csl = slice(jt * 512 + qc * 128, jt * 512 + (qc + 1) * 128)
                    nc.vector.tensor_copy(OT[:, hp, csl], tp[:])

            # ---- emit passes ----
            for p in range(repeat):
                par = p % 2
                if p == 0:
                    xc0 = issue_xc_dma(0)
                    for t in proj_thunks(0, 0, xc0):
                        t()
                for jt in range(NJT):
                    if jt + 1 < NJT:
                        xcn = issue_xc_dma(jt + 1)
                        fillers.extend(proj_thunks(jt + 1, par, xcn))
                    elif p + 1 < repeat:
                        xcn = issue_xc_dma(0)
                        fillers.extend(proj_thunks(0, (p + 1) % 2, xcn))
                    if jt >= 1:
                        fillers.extend(outproj_thunks(jt - 1))
                    elif p >= 1:
                        fillers.extend(outproj_thunks(3))
                    set_pace(4 * (4 * (jt + 1) + 4))
                    for hp in range(4):
                        strip(hp, jt, par)
            # tail: remaining projections of the last pass
            while fillers:
                fillers.popleft()()
            for t in outproj_thunks(2):
                t()
            for t in outproj_thunks(3):
                t()
            if debug:
                nc.sync.dma_start(qt_d[:].rearrange("p (a b) -> p a b", a=4),
                                  QT[:])
                nc.sync.dma_start(kt_d[:].rearrange("p (a b) -> p a b", a=4),
                                  KTbuf[(repeat - 1) % 2][:])
                nc.sync.dma_start(
                    v_d[:].rearrange("p (a b c) -> p a b c", a=NKB, b=HPC),
                    Vbuf[(repeat - 1) % 2][:])
                nc.sync.dma_start(ot_d[:].rearrange("p (a b) -> p a b", a=4),
                                  OT[:])
                nc.sync.dma_start(et_d[:].rearrange("p (a b) -> p a b", a=2),
                                  et_tiles[0][:])

    nc.compile()
    return nc


def _get_nc():
    if "nc" not in _STATE:
        _STATE["nc"] = _build_nc()
    return _STATE["nc"]


def kernel(x, in_w, in_b, out_w, out_b):
    from concourse.bass_utils import run_bass_kernel_spmd

    bf = ml_dtypes.bfloat16
    x = np.asarray(x, dtype=np.float32)
    in_w = np.asarray(in_w, dtype=np.float32)
    in_b = np.asarray(in_b, dtype=np.float32)
    out_w = np.asarray(out_w, dtype=np.float32)
    out_b = np.asarray(out_b, dtype=np.float32)

    nc = _get_nc()

    in_maps = []
    for c in range(NCORES):
        b, hg = c // 2, c % 2
        hsl = slice(hg * HPC * HD, (hg + 1) * HPC * HD)  # 512 cols of each section
        wq = in_w[:, 0:D][:, hsl]
        wk = in_w[:, D:2 * D][:, hsl]
        wv_ = in_w[:, 2 * D:3 * D][:, hsl]
        bq = in_b[0:D][hsl]
        bk = in_b[D:2 * D][hsl]
        bv_ = in_b[2 * D:3 * D][hsl]
        in_maps.append({
            "xT": np.ascontiguousarray(x[b].T).astype(bf),
            "wqk": np.ascontiguousarray(
                np.concatenate([wq, wk], axis=1)).astype(bf),
            "wv": np.ascontiguousarray(wv_).astype(bf),
            "bqk": np.ascontiguousarray(
                np.concatenate([bq, bk]).reshape(8, 128).T).astype(np.float32),
            "bv": np.ascontiguousarray(bv_.reshape(1, -1)).astype(bf),
            "w2": np.ascontiguousarray(out_w[hsl, :]).astype(bf),
        })

    trace = bool(int(os.environ.get("KERNEL_TRACE", "0")))
    if not trace:
        # the axon NTFF profile hook is absent in this container; make sure a
        # stray BASS_TRACE=1 in the environment can't route us into it
        os.environ["BASS_NEVER_TRACE"] = "1"
    res = run_bass_kernel_spmd(
        nc, in_maps, core_ids=list(range(NCORES)), trace=trace,
    )
    _STATE["last_result"] = res
    _STATE["last_in_maps"] = in_maps

    y = np.zeros((B, L, D), dtype=np.float32)
    for c in range(NCORES):
        y[c // 2] += res.results[c]["out"]
    y += out_b[None, None, :]
    return y


# revision 10
# speedup vs baseline: 3.1855x; 1.6235x over previous
"""Causal self-attention (B=4, L=2048, D=1024, H=16, HD=64) on 8 TRN2 cores.

Sharding: 8 shards = 4 batches x 2 head-groups (8 heads each). Each core:
  - QKV projection for its 8 heads (Q^T/K^T in [HD, L] layout, V in [L, HD])
  - causal attention per head pair (even head on partitions 0-63, odd on
    64-127), softmax without max-subtraction (logits are small by
    construction), row sums via a ones-column appended to V
  - partial output projection with its 512 rows of out_w
Host sums the two partials per batch and adds out_b.

Single fused pipeline per 512-query tile jt:
  S^T = K^T q  (PSUM, causal mask folded in as an identity x (-3000 *
  strict-lower) matmul joining the accumulation), one exp per key block
  covering both heads of the pair, then U = E^T V_aug in [q, d] layout
  (N=65 with the ones column), per-partition normalize on DVE, PE
  transpose back to [d, q] for the output projection.

Projection / output-projection work is emitted as filler thunks between
attention key blocks so the PE never idles (the PE clock drops ~2x after
any idle gap). K^T and V are double-buffered across repeat passes so
consecutive passes in the timing NEFF overlap.
"""

import os
from collections import deque

import numpy as np
import ml_dtypes

B, L, D, H, HD = 4, 2048, 1024, 16, 64
HPC = 8           # heads per core
NCORES = 8
NKB = L // 128    # key blocks of 128
NJT = L // 512    # query tiles of 512

_STATE = {}


def _build_nc(repeat=1):
    import concourse.bass as bass
    import concourse.mybir as mybir
    import concourse.tile as tile
    from concourse import bacc
    from concourse.masks import make_lower_triangular, make_identity

    f32 = mybir.dt.float32
    bf16 = mybir.dt.bfloat16
    AF = mybir.ActivationFunctionType

    nc = bacc.Bacc(None, target_bir_lowering=False)

    xT = nc.dram_tensor("xT", [D, L], bf16, kind="ExternalInput")
    wqk = nc.dram_tensor("wqk", [D, 2 * HPC * HD], bf16, kind="ExternalInput")
    wv = nc.dram_tensor("wv", [D, HPC * HD], bf16, kind="ExternalInput")
    bqk = nc.dram_tensor("bqk", [128, 8], f32, kind="ExternalInput")
    bv = nc.dram_tensor("bv", [1, HPC * HD], bf16, kind="ExternalInput")
    w2 = nc.dram_tensor("w2", [HPC * HD, D], bf16, kind="ExternalInput")
    out = nc.dram_tensor("out", [L, D], f32, kind="ExternalOutput")
    debug = bool(int(os.environ.get("KDEBUG", "0")))
    if debug:
        qt_d = nc.dram_tensor("qt_d", [128, 4 * L], bf16, kind="ExternalOutput")
        kt_d = nc.dram_tensor("kt_d", [128, 4 * L], bf16, kind="ExternalOutput")
        v_d = nc.dram_tensor("v_d", [128, NKB * HPC * 66], bf16,
                             kind="ExternalOutput")
        ot_d = nc.dram_tensor("ot_d", [128, 4 * L], bf16, kind="ExternalOutput")
        et_d = nc.dram_tensor("et_d", [128, 2 * 512], bf16, kind="ExternalOutput")

    KO = D // 128  # contraction blocks for the projections
    xTr = xT.rearrange("(ko p) n -> p ko n", p=128)

    with tile.TileContext(nc) as tc:
        with (
            tc.tile_pool(name="const", bufs=1) as cpool,
            tc.tile_pool(name="weights", bufs=1) as wpool,
            tc.tile_pool(name="resident", bufs=1) as rpool,
            tc.tile_pool(name="xc", bufs=2) as xcpool,
            tc.tile_pool(name="ework", bufs=1) as epool,
            tc.tile_pool(name="nwork", bufs=3) as npool,
            tc.tile_pool(name="ywork", bufs=3) as ypool,
            tc.tile_pool(name="ps_s", bufs=2, space="PSUM") as ps_s,
            tc.tile_pool(name="ps_u", bufs=2, space="PSUM") as ps_u,
            tc.tile_pool(name="ps_mm", bufs=2, space="PSUM") as ps_mm,
        ):
            ones = cpool.tile([1, 128], bf16)
            nc.vector.memset(ones[:], 1.0)
            ident = cpool.tile([128, 128], bf16)
            make_identity(nc, ident[:])
            maskneg = cpool.tile([128, 128], bf16)
            make_lower_triangular(nc, maskneg[:], val=-3000.0, diag=False)

            wqk_sb = wpool.tile([128, KO, 2 * HPC * HD], bf16)
            nc.sync.dma_start(wqk_sb[:], wqk.rearrange("(ko p) m -> p ko m", p=128))
            wv_sb = wpool.tile([128, KO, HPC * HD], bf16)
            nc.sync.dma_start(wv_sb[:], wv.rearrange("(ko p) m -> p ko m", p=128))
            bqk_sb = wpool.tile([128, 8], f32)
            nc.sync.dma_start(bqk_sb[:], bqk[:])
            bv_sb = wpool.tile([1, HPC * HD], bf16)
            nc.sync.dma_start(bv_sb[:], bv[:])
            w2_sb = wpool.tile([128, 4, D], bf16)
            nc.sync.dma_start(w2_sb[:], w2.rearrange("(o p) n -> p o n", p=128))

            # Q^T packed as head pairs: head h at partitions (h%2)*64..+64 of
            # block h//2.  K^T and V double-buffered by pass parity so the
            # next pass's projections can overlap this pass's last strips.
            QT = rpool.tile([128, 4, L], bf16)
            OT = rpool.tile([128, 4, L], bf16)
            KTbuf = [rpool.tile([128, 4, L], bf16, name=f"KT{i}") for i in (0, 1)]
            # V with a ones column at index 64 (col 65 is alignment padding).
            Vbuf = [rpool.tile([128, NKB, HPC, 66], bf16, name=f"V{i}")
                    for i in (0, 1)]
            for Vb in Vbuf:
                nc.vector.memset(Vb[:, :, :, 64:66], 0.0)
                nc.vector.memset(Vb[:, :, :, 64:65], 1.0)

            et_tiles = [
                epool.tile([128, 2, 512], bf16, tag=f"et{kb}", name=f"et{kb}")
                for kb in range(NKB)
            ]
            skel = bool(int(os.environ.get("KSKEL", "0")))
            if skel:
                for t in et_tiles:
                    nc.vector.memset(t[:], 1.0)

            fillers = deque()
            pace = {"credit": 0.0, "rate": 0.0}

            def set_pace(slots):
                pace["rate"] = len(fillers) / max(1, slots)
                pace["credit"] = 0.0

            def pop_filler():
                """Paced pop: spread the queued thunks over the set's slots."""
                pace["credit"] += pace["rate"]
                while pace["credit"] >= 1.0 and fillers:
                    pace["credit"] -= 1.0
                    fillers.popleft()()

            def issue_xc_dma(jt):
                xc = xcpool.tile([128, KO, 512], bf16, tag="xc", name="xc")
                nc.sync.dma_start(xc[:], xTr[:, :, jt * 512:(jt + 1) * 512])
                return xc

            def proj_thunks(jt, par, xc):
                """QKV projection for query/key tile jt into parity-par bufs."""
                KT, V = KTbuf[par], Vbuf[par]
                sl = slice(jt * 512, (jt + 1) * 512)
                thunks = []

                def qk(mb):
                    def t():
                        tq = ps_mm.tile([128, 512], f32, tag="mm512", name="tq")
                        for ko in range(KO):
                            nc.tensor.matmul(
                                tq[:],
                                wqk_sb[:, ko, mb * 128:(mb + 1) * 128],
                                xc[:, ko, :],
                                start=(ko == 0),
                                stop=(ko == KO - 1),
                            )
                        dst = QT[:, mb, sl] if mb < 4 else KT[:, mb - 4, sl]
                        nc.vector.tensor_scalar_add(dst, tq[:], bqk_sb[:, mb:mb + 1])
                    return t

                def vproj(qb):
                    def t():
                        tv = ps_mm.tile([128, 512], f32, tag="mm512", name="tv")
                        for ko in range(KO):
                            nc.tensor.matmul(
                                tv[:],
                                xc[:, ko, qb * 128:(qb + 1) * 128],
                                wv_sb[:, ko, :],
                                start=(ko == 0),
                                stop=False,
                            )
                        nc.tensor.matmul(
                            tv[:], ones[0:1, :], bv_sb[0:1, :],
                            start=False, stop=True,
                        )
                        nc.vector.tensor_copy(
                            V[:, jt * 4 + qb, :, 0:64],
                            tv.rearrange("p (h e) -> p h e", e=HD),
                        )
                    return t

                for mb in range(8):
                    thunks.append(qk(mb))
                for qb in range(4):
                    thunks.append(vproj(qb))
                return thunks

            def outproj_thunks(jt):
                """Output projection for the 512 queries of tile jt."""
                thunks = []

                def yproj(qb, nb):
                    def t():
                        y = ps_mm.tile([128, 512], f32, tag="mm512", name="y")
                        for hp in range(4):
                            nc.tensor.matmul(
                                y[:],
                                OT[:, hp, qb * 128:(qb + 1) * 128],
                                w2_sb[:, hp, nb * 512:(nb + 1) * 512],
                                start=(hp == 0),
                                stop=(hp == 3),
                            )
                        ysb = ypool.tile([128, 512], f32, tag="ysb", name="ysb")
                        nc.vector.tensor_copy(ysb[:], y[:])
                        nc.sync.dma_start(
                            out[qb * 128:(qb + 1) * 128, nb * 512:(nb + 1) * 512],
                            ysb[:],
                        )
                    return t

                for qb in range(jt * 4, (jt + 1) * 4):
                    for nb in range(2):
                        thunks.append(yproj(qb, nb))
                return thunks

            def strip(hp, jt, par):
                """Attention for head pair hp, query tile jt."""
                KT, V = KTbuf[par], Vbuf[par]
                nkb = 4 * (jt + 1)
                # S + exp phase
                for kb in range(nkb):
                    qoff = max(0, (kb - 4 * jt) * 128)
                    qsl = slice(jt * 512 + qoff, (jt + 1) * 512)
                    ksl = slice(kb * 128, (kb + 1) * 128)
                    diag = kb >= 4 * jt
                    sp = ps_s.tile([128, 2, 512], f32, tag="sp", name="sp")
                    for side in (0, 1):
                        p0, p1 = side * 64, side * 64 + 64
                        nc.tensor.matmul(
                            sp[:, side, qoff:],
                            KT[p0:p1, hp, ksl],
                            QT[p0:p1, hp, qsl],
                            start=True,
                            stop=not diag,
                        )
                        if diag:
                            nc.tensor.matmul(
                                sp[:, side, qoff:qoff + 128],
                                ident[:],
                                maskneg[:],
                                start=False,
                                stop=True,
                            )
                    if not skel:
                        nc.scalar.activation(
                            et_tiles[kb][:, :, qoff:], sp[:, :, qoff:],
                            AF.Exp, scale=0.125,
                        )
                    pop_filler()
                # U phase: [q, d] layout, ones column gives row sums at col 64.
                # The DVE normalize runs while the PE continues with the next
                # chunk's U matmuls; transposes are deferred to the end so the
                # PE never waits on the DVE chain.
                oqds = []
                for qc in range(4):
                    last = 4 * jt + qc
                    for side in (0, 1):
                        u = ps_u.tile([128, 66], f32, tag="u", name="u")
                        for kb in range(last + 1):
                            nc.tensor.matmul(
                                u[:, 0:65],
                                et_tiles[kb][:, side, qc * 128:(qc + 1) * 128],
                                V[:, kb, 2 * hp + side, 0:65],
                                start=(kb == 0),
                                stop=(kb == last),
                            )
                        rcp = npool.tile([128, 1], f32, tag="rcp", name="rcp")
                        nc.vector.reciprocal(rcp[:], u[:, 64:65])
                        oqd = npool.tile([128, 64], bf16, tag="oqd",
                                         bufs=8, name="oqd")
                        nc.vector.tensor_scalar_mul(oqd[:], u[:, 0:64], rcp[:])
                        oqds.append(oqd)
                    pop_filler()
                for qc in range(4):
                    tp = ps_mm.tile([128, 128], bf16, tag="mm512", name="tp")
                    for side in (0, 1):
                        nc.tensor.transpose(
                            tp[side * 64:side * 64 + 64, :],
                            oqds[2 * qc + side][:], ident[:],
                            tile_position=(0, side * 64),
                        )
                    csl = slice(jt * 512 + qc * 128, jt * 512 + (qc + 1) * 128)
                    nc.vector.tensor_copy(OT[:, hp, csl], tp[:])

            # ---- emit passes ----
            for p in range(repeat):
                par = p % 2
                if p == 0:
                    xc0 = issue_xc_dma(0)
                    for t in proj_thunks(0, 0, xc0):
                        t()
                for jt in range(NJT):
                    if jt + 1 < NJT:
                        xcn = issue_xc_dma(jt + 1)
                        fillers.extend(proj_thunks(jt + 1, par, xcn))
                    elif p + 1 < repeat:
                        xcn = issue_xc_dma(0)
                        fillers.extend(proj_thunks(0, (p + 1) % 2, xcn))
                    if jt >= 1:
                        fillers.extend(outproj_thunks(jt - 1))
                    elif p >= 1:
                        fillers.extend(outproj_thunks(3))
                    set_pace(4 * (4 * (jt + 1) + 4))
                    for hp in range(4):
                        strip(hp, jt, par)
            # tail: remaining projections of the last pass
            while fillers:
                fillers.popleft()()
            for t in outproj_thunks(2):
                t()
            for t in outproj_thunks(3):
                t()
            if debug:
                nc.sync.dma_start(qt_d[:].rearrange("p (a b) -> p a b", a=4),
                                  QT[:])
                nc.sync.dma_start(kt_d[:].rearrange("p (a b) -> p a b", a=4),
                                  KTbuf[(repeat - 1) % 2][:])
                nc.sync.dma_start(
                    v_d[:].rearrange("p (a b c) -> p a b c", a=NKB, b=HPC),
                    Vbuf[(repeat - 1) % 2][:])
                nc.sync.dma_start(ot_d[:].rearrange("p (a b) -> p a b", a=4),
                                  OT[:])
                nc.sync.dma_start(et_d[:].rearrange("p (a b) -> p a b", a=2),
                                  et_tiles[0][:])

    nc.compile()
    return nc


def _get_nc():
    if "nc" not in _STATE:
        _STATE["nc"] = _build_nc()
    return _STATE["nc"]


def kernel(x, in_w, in_b, out_w, out_b):
    from concourse.bass_utils import run_bass_kernel_spmd

    bf = ml_dtypes.bfloat16
    x = np.asarray(x, dtype=np.float32)
    in_w = np.asarray(in_w, dtype=np.float32)
    in_b = np.asarray(in_b, dtype=np.float32)
    out_w = np.asarray(out_w, dtype=np.float32)
    out_b = np.asarray(out_b, dtype=np.float32)

    nc = _get_nc()

    in_maps = []
    for c in range(NCORES):
        b, hg = c // 2, c % 2
        hsl = slice(hg * HPC * HD, (hg + 1) * HPC * HD)  # 512 cols of each section
        wq = in_w[:, 0:D][:, hsl]
        wk = in_w[:, D:2 * D][:, hsl]
        wv_ = in_w[:, 2 * D:3 * D][:, hsl]
        bq = in_b[0:D][hsl]
        bk = in_b[D:2 * D][hsl]
        bv_ = in_b[2 * D:3 * D][hsl]
        in_maps.append({
            "xT": np.ascontiguousarray(x[b].T).astype(bf),
            "wqk": np.ascontiguousarray(
                np.concatenate([wq, wk], axis=1)).astype(bf),
            "wv": np.ascontiguousarray(wv_).astype(bf),
            "bqk": np.ascontiguousarray(
                np.concatenate([bq, bk]).reshape(8, 128).T).astype(np.float32),
            "bv": np.ascontiguousarray(bv_.reshape(1, -1)).astype(bf),
            "w2": np.ascontiguousarray(out_w[hsl, :]).astype(bf),
        })

    trace = bool(int(os.environ.get("KERNEL_TRACE", "0")))
    if not trace:
        # the axon NTFF profile hook is absent in this container; make sure a
        # stray BASS_TRACE=1 in the environment can't route us into it
        os.environ["BASS_NEVER_TRACE"] = "1"
    res = run_bass_kernel_spmd(
        nc, in_maps, core_ids=list(range(NCORES)), trace=trace,
    )
    _STATE["last_result"] = res
    _STATE["last_in_maps"] = in_maps

    y = np.zeros((B, L, D), dtype=np.float32)
    for c in range(NCORES):
        y[c // 2] += res.results[c]["out"]
    y += out_b[None, None, :]
    return y
